# revision 7
# baseline (speedup 1.0000x reference)
"""Trainium2 Bass kernel for a 3-layer spiking neural net (DSNN).

Reference semantics (per timestep t = 1..99, batch 512):
    l0: nm0 = 0.9*mem0 + x@W0;   s0 = nm0>1;  mem0 = nm0 if nm0<=1 else 0
    l1: ns1 = 0.95*syn1 + s0@W1; nm1 = 0.9*mem1 + ns1; s1 = nm1>1; reset
    l2: ns2 = 0.95*syn2 + s1@W2; mem2 = 0.9*mem2 + ns2   (linear, no reset)
    output: mem2 at t=99.

Kernel strategy (data-parallel: batch 512 -> 64 per core, 8 cores, no
collectives):
  - x@W0 is loop-invariant: computed once (PE transpose of x + 8 matmuls).
  - State kept hidden-major: [128 partitions, 4 chunks, 64 batch].
  - Layer-2 is linear, so the output collapses to
        mem2(99) = sum_u c_u * (s1(u) @ W2),  c_u = (a^(100-u)-b^(100-u))/(a-b)
    accumulated entirely in one PSUM bank via tiny matmuls against
    c_u-prescaled W2 (split hi/lo bf16 for fp32-level accuracy).
  - Time is processed in blocks of 4 steps. Per block: the layer-0
    elementwise recurrence emits spikes s0 (fp32), one batched float32r
    matmul produces z1 = s0@W1 for the block (full-rate fp32 on the PE),
    then the layer-1 recurrence consumes z1 straight from PSUM.
  - Engine split per step: GpSimd: nm0 update + s1 compare; VectorE:
    resets + syn/mem updates; ScalarE: s0 via Sign+Relu; PE: matmuls.
    Software-pipelined so layer-0 runs 2 block-slots ahead of layer-1.
"""

import numpy as np

import bass_rust
import concourse.bass as bass
import concourse.mybir as mybir
from concourse.tile import TileContext

F32 = mybir.dt.float32
F32R = mybir.dt.float32r
BF16 = mybir.dt.bfloat16
Op = mybir.AluOpType
Act = mybir.ActivationFunctionType

ALPHA = 0.95
BETA = 0.9
T = 99            # update steps (SIM_TIME - 1)
TBLK = 4          # timesteps per block (psum z1 tile = 2 banks)
B_LOC = 64        # batch per core
N_CORES = 8
H_IN, H1, H2, N_OUT = 256, 512, 512, 4
KC1 = H1 // 128   # 4 hidden chunks

_BLOCKS = []
_t = 1
while _t <= T:
    _BLOCKS.append((_t, min(TBLK, T - _t + 1)))
    _t += TBLK
NBLK = len(_BLOCKS)


def c_coeffs():
    """c_u for u=1..99 with mem2(99) = sum_u c_u * z2(u)."""
    u = np.arange(1, T + 1, dtype=np.float64)
    c = (ALPHA ** (T + 1 - u) - BETA ** (T + 1 - u)) / (ALPHA - BETA)
    return c.astype(np.float32)


def split_multi_waits(nc, max_waits=1):
    """This container's walrus rejects >1 semaphore wait per instruction;
    hoist excess waits onto same-engine NoOps inserted just before."""
    n = 0
    cnt = [0]
    for f in nc.m.functions:
        for bb in f.blocks:
            out = []
            changed = False
            for inst in bb.instructions:
                si = inst.sync_info
                waits = list(si.on_wait) if si is not None and si.on_wait else []
                if len(waits) > max_waits:
                    changed = True
                    n += 1
                    head, tail = waits[:-max_waits], waits[-max_waits:]
                    for i in range(0, len(head), max_waits):
                        nop = bass_rust.InstNoOp(
                            name=f"I-waitsplit-{cnt[0]}", ins=[], outs=[]
                        )
                        cnt[0] += 1
                        nop.engine = inst.engine
                        nop.sync_info = bass_rust.SyncInfo(
                            on_wait=head[i : i + max_waits], on_update=[]
                        )
                        out.append(nop)
                    si.on_wait = tail
                    inst.sync_info = si
                out.append(inst)
            if changed:
                bb.instructions = out
    return n


def build_nc():
    nc = bass.Bass(target_bir_lowering=False)

    x_d = nc.dram_tensor("inputs", [B_LOC, H_IN], F32, kind="ExternalInput")
    w0_d = nc.dram_tensor("W0", [H_IN, H1], F32, kind="ExternalInput")
    w1_d = nc.dram_tensor("W1", [H1, H2], F32, kind="ExternalInput")
    w2_d = nc.dram_tensor("W2", [H2, N_OUT], F32, kind="ExternalInput")
    cv_d = nc.dram_tensor("cvec", [128, T], F32, kind="ExternalInput")
    y_d = nc.dram_tensor("out", [B_LOC, N_OUT], F32, kind="ExternalOutput")

    from concourse.masks import make_identity

    with TileContext(nc) as tc:
        with (
            tc.tile_pool(name="consts", bufs=1) as consts,
            tc.tile_pool(name="tmp", bufs=1) as tmp,
            tc.tile_pool(name="state", bufs=1) as state,
            tc.tile_pool(name="s0p", bufs=2) as s0p,
            tc.tile_pool(name="s1p", bufs=2) as s1p,
            tc.tile_pool(name="psA", bufs=1, space="PSUM") as psA,
            tc.tile_pool(name="psZ", bufs=2, space="PSUM") as psZ,
        ):
            # ---------------- setup: weight/const loads -----------------
            w0_sb = consts.tile([128, 2, H1], F32)
            nc.sync.dma_start(
                out=w0_sb, in_=w0_d.rearrange("(kc p) j -> p kc j", p=128)
            )
            w1_sb = consts.tile([128, KC1, H2], F32)
            nc.sync.dma_start(
                out=w1_sb, in_=w1_d.rearrange("(kc p) j -> p kc j", p=128)
            )
            w2_sb = consts.tile([128, KC1, N_OUT], F32)
            nc.sync.dma_start(
                out=w2_sb, in_=w2_d.rearrange("(kc p) o -> p kc o", p=128)
            )
            cb = consts.tile([128, T], F32)
            nc.sync.dma_start(out=cb, in_=cv_d[:, :])
            x_sb = consts.tile([B_LOC, H_IN], F32)
            nc.sync.dma_start(out=x_sb, in_=x_d[:, :])

            bias_m1 = consts.tile([128, 1], F32)
            nc.vector.memset(bias_m1, -1.0)
            ident = consts.tile([B_LOC, B_LOC], F32)
            make_identity(nc, ident)

            # W2c[p, kc, t, o] = W2[p, kc, o] * c[t], split hi+lo bf16.
            w2c_f = tmp.tile([128, KC1, T, N_OUT], F32)
            nc.vector.tensor_tensor(
                out=w2c_f,
                in0=w2_sb.unsqueeze(2).to_broadcast([128, KC1, T, N_OUT]),
                in1=cb.unsqueeze(1).unsqueeze(3).to_broadcast([128, KC1, T, N_OUT]),
                op=Op.mult,
            )
            w2c_hi = consts.tile([128, KC1, T, N_OUT], BF16)
            nc.vector.tensor_copy(w2c_hi, w2c_f)
            hi32 = tmp.tile([128, KC1, T, N_OUT], F32)
            nc.vector.tensor_copy(hi32, w2c_hi)
            rem = tmp.tile([128, KC1, T, N_OUT], F32)
            nc.vector.tensor_tensor(out=rem, in0=w2c_f, in1=hi32, op=Op.subtract)
            w2c_lo = consts.tile([128, KC1, T, N_OUT], BF16)
            nc.vector.tensor_copy(w2c_lo, rem)

            # ------------- Z0 = (x @ W0), stored transposed -------------
            # xT via PE transpose, then Z0T[p, jc, b] = W0.T @ xT.
            xt_sb = consts.tile([128, 2, B_LOC], F32)
            for kc in range(2):
                xt_ps = psA.tile([128, B_LOC], F32)
                nc.tensor.transpose(
                    xt_ps, x_sb[:, kc * 128 : (kc + 1) * 128], ident
                )
                nc.vector.tensor_copy(xt_sb[:, kc, :], xt_ps)
            w1r = consts.tile([128, KC1, H2], F32R)
            nc.vector.tensor_copy(w1r, w1_sb)
            z0t = consts.tile([128, KC1, B_LOC], F32)
            z0_ps = psA.tile([128, KC1, B_LOC], F32)
            for jc in range(KC1):
                for kc in range(2):
                    nc.tensor.matmul(
                        z0_ps[:, jc, :],
                        w0_sb[:, kc, jc * 128 : (jc + 1) * 128],
                        xt_sb[:, kc, :],
                        start=(kc == 0),
                        stop=(kc == 1),
                    )
            nc.vector.tensor_copy(z0t, z0_ps)

            # output accumulator (one PSUM bank, lives for the whole loop)
            out_ps = psA.tile([N_OUT, B_LOC], F32)

            # ---------------- state tiles ----------------
            m0 = state.tile([128, KC1, B_LOC], F32)
            bm0 = state.tile([128, KC1, B_LOC], F32)
            nm0 = state.tile([128, KC1, B_LOC], F32)
            sg = state.tile([128, KC1, B_LOC], F32)
            mem1 = state.tile([128, KC1, B_LOC], F32)
            nm1 = state.tile([128, KC1, B_LOC], F32)
            syn1a = state.tile([128, KC1, B_LOC], F32)
            syn1b = state.tile([128, KC1, B_LOC], F32)
            syn = [syn1a, syn1b]
            for st in (m0, mem1, syn1a, syn1b):
                nc.vector.memset(st, 0.0)

            s0_blks = [None] * NBLK
            s1_blks = [None] * NBLK
            z1_blks = [None] * NBLK

            n_w2c = 2 * KC1 * T
            w2c_done = [0]

            def l0_step(blk, i):
                t0, tb = _BLOCKS[blk]
                t = t0 + i
                # (a1) bm0 = 0.9*m0              [ScalarE]
                nc.scalar.mul(out=bm0, in_=m0, mul=BETA)
                # (a2) nm0 = bm0 + z0            [GpSimd]
                nc.gpsimd.tensor_tensor(out=nm0, in0=bm0, in1=z0t, op=Op.add)
                # (b) m0 = (nm0<=1)*nm0          [VectorE]
                nc.vector.scalar_tensor_tensor(
                    out=m0, in0=nm0, scalar=1.0, in1=nm0,
                    op0=Op.is_le, op1=Op.mult,
                )
                # (c) s0 = relu(sign(nm0-1))     [ScalarE x2]
                nc.scalar.activation(
                    out=sg, in_=nm0, func=Act.Sign, bias=bias_m1[:, 0:1], scale=1.0
                )
                nc.scalar.activation(
                    out=s0_blks[blk][:, :, i, :], in_=sg, func=Act.Relu
                )

            def l1_step(blk, i):
                t0, tb = _BLOCKS[blk]
                t = t0 + i
                p = t % 2
                # (d) syn = 0.95*syn_prev + z1   [VectorE, z1 from PSUM]
                nc.vector.scalar_tensor_tensor(
                    out=syn[p], in0=syn[1 - p], scalar=ALPHA,
                    in1=z1_blks[blk][:, :, i, :],
                    op0=Op.mult, op1=Op.add,
                )
                # (e) nm1 = 0.9*mem1 + syn       [VectorE]
                nc.vector.scalar_tensor_tensor(
                    out=nm1, in0=mem1, scalar=BETA, in1=syn[p],
                    op0=Op.mult, op1=Op.add,
                )
                # (f) mem1 = (nm1<=1)*nm1        [VectorE]
                nc.vector.scalar_tensor_tensor(
                    out=mem1, in0=nm1, scalar=1.0, in1=nm1,
                    op0=Op.is_le, op1=Op.mult,
                )

            def l1_spike(blk, i):
                # (g) s1 = (nm1>1) -> bf16       [GpSimd]
                nc.gpsimd.tensor_scalar(
                    out=s1_blks[blk][:, :, i, :], in0=nm1,
                    scalar1=1.0, scalar2=None, op0=Op.is_gt,
                )

            def w1_mms(blk):
                t0, tb = _BLOCKS[blk]
                for jc in range(KC1):
                    for kc in range(KC1):
                        nc.tensor.matmul(
                            z1_blks[blk][:, jc, 0:tb, :],
                            w1r[:, kc, jc * 128 : (jc + 1) * 128],
                            s0_blks[blk][:, kc, 0:tb, :],
                            start=(kc == 0),
                            stop=(kc == KC1 - 1),
                        )

            def w2c_mms(blk):
                t0, tb = _BLOCKS[blk]
                for i in range(tb):
                    t = t0 + i
                    for kc in range(KC1):
                        for wt in (w2c_hi, w2c_lo):
                            nc.tensor.matmul(
                                out_ps[:, :],
                                wt[:, kc, t - 1, :],
                                s1_blks[blk][:, kc, i, :],
                                start=(w2c_done[0] == 0),
                                stop=(w2c_done[0] == n_w2c - 1),
                                skip_group_check=True,
                            )
                            w2c_done[0] += 1

            # ------------- software-pipelined main loop -------------
            # slot sl: L0 on block sl, W1 matmul on block sl-1,
            #          L1 + W2c on block sl-2.
            for sl in range(NBLK + 2):
                lead = sl if sl < NBLK else None          # L0 block
                mmb = sl - 1 if 0 <= sl - 1 < NBLK else None
                lag = sl - 2 if sl - 2 >= 0 else None     # L1 block

                if lead is not None:
                    s0_blks[lead] = s0p.tile([128, KC1, TBLK, B_LOC], F32R, name="s0blk", tag="s0blk")
                    s1_blks[lead] = s1p.tile([128, KC1, TBLK, B_LOC], BF16, name="s1blk", tag="s1blk")
                if mmb is not None:
                    z1_blks[mmb] = psZ.tile([128, KC1, TBLK, B_LOC], F32, name="z1blk", tag="z1blk")
                    w1_mms(mmb)

                tb0 = _BLOCKS[lead][1] if lead is not None else 0
                tb2 = _BLOCKS[lag][1] if lag is not None else 0
                for i in range(TBLK):
                    if lead is not None and i < tb0:
                        l0_step(lead, i)
                    if lag is not None and 0 <= i - 1 < tb2:
                        l1_spike(lag, i - 1)
                    if lag is not None and i < tb2:
                        l1_step(lag, i)
                if lag is not None:
                    l1_spike(lag, tb2 - 1)
                    w2c_mms(lag)
                    z1_blks[lag] = None
                    s0_blks[lag] = None
                    s1_blks[lag] = None

            # ---------------- epilogue ----------------
            out_sb = consts.tile([N_OUT, B_LOC], F32)
            nc.vector.tensor_copy(out_sb, out_ps)
            nc.sync.dma_start(out=y_d[:, :].transpose([1, 0]), in_=out_sb)

    return nc


_CACHE = {}


def _get_nc():
    if "nc" not in _CACHE:
        nc = build_nc()
        split_multi_waits(nc)
        _CACHE["nc"] = nc
    return _CACHE["nc"]


def make_in_maps(inputs, W0, W1, W2):
    cv = np.ascontiguousarray(
        np.broadcast_to(c_coeffs()[None, :], (128, T))
    ).astype(np.float32)
    maps = []
    for c in range(N_CORES):
        maps.append(
            {
                "inputs": np.ascontiguousarray(
                    inputs[c * B_LOC : (c + 1) * B_LOC]
                ).astype(np.float32),
                "W0": np.ascontiguousarray(W0).astype(np.float32),
                "W1": np.ascontiguousarray(W1).astype(np.float32),
                "W2": np.ascontiguousarray(W2).astype(np.float32),
                "cvec": cv,
            }
        )
    return maps


def kernel(inputs, W0, W1, W2):
    import os

    os.environ["BASS_NEVER_TRACE"] = "1"
    from concourse.bass_utils import run_bass_kernel_spmd

    nc = _get_nc()
    in_maps = make_in_maps(inputs, W0, W1, W2)
    res = run_bass_kernel_spmd(nc, in_maps, core_ids=list(range(N_CORES)))
    return np.concatenate([r["out"] for r in res.results], axis=0)


# revision 15
# speedup vs baseline: 1.0065x; 1.0065x over previous
"""Trainium2 Bass kernel for a 3-layer spiking neural net (DSNN).

Reference semantics (per timestep t = 1..99, batch 512):
    l0: nm0 = 0.9*mem0 + x@W0;   s0 = nm0>1;  mem0 = nm0 if nm0<=1 else 0
    l1: ns1 = 0.95*syn1 + s0@W1; nm1 = 0.9*mem1 + ns1; s1 = nm1>1; reset
    l2: ns2 = 0.95*syn2 + s1@W2; mem2 = 0.9*mem2 + ns2   (linear, no reset)
    output: mem2 at t=99.

Kernel strategy (data-parallel: batch 512 -> 64 per core, 8 cores, no
collectives):
  - x@W0 is loop-invariant: computed once (PE transpose of x + 8 matmuls).
  - State kept hidden-major: [128 partitions, 4 chunks, 64 batch].
  - Layer-2 is linear, so the output collapses to
        mem2(99) = sum_u c_u * (s1(u) @ W2),  c_u = (a^(100-u)-b^(100-u))/(a-b)
    accumulated entirely in one PSUM bank via tiny matmuls against
    c_u-prescaled W2 (split hi/lo bf16 for fp32-level accuracy).
  - Time is processed in blocks of 4 steps. Per block: the layer-0
    elementwise recurrence emits spikes s0 (fp32), one batched float32r
    matmul produces z1 = s0@W1 for the block (full-rate fp32 on the PE),
    then the layer-1 recurrence consumes z1 straight from PSUM.
  - Engine split per step: GpSimd: nm0 update + s1 compare; VectorE:
    resets + syn/mem updates; ScalarE: s0 via Sign+Relu; PE: matmuls.
    Software-pipelined so layer-0 runs 2 block-slots ahead of layer-1.
"""

import numpy as np

import bass_rust
import concourse.bass as bass
import concourse.mybir as mybir
from concourse.tile import TileContext

F32 = mybir.dt.float32
F32R = mybir.dt.float32r
BF16 = mybir.dt.bfloat16
Op = mybir.AluOpType
Act = mybir.ActivationFunctionType

ALPHA = 0.95
BETA = 0.9
T = 99            # update steps (SIM_TIME - 1)
TBLK = 4          # timesteps per block (psum z1 tile = 2 banks)
B_LOC = 64        # batch per core
N_CORES = 8
H_IN, H1, H2, N_OUT = 256, 512, 512, 4
KC1 = H1 // 128   # 4 hidden chunks

_BLOCKS = []
_t = 1
while _t <= T:
    _BLOCKS.append((_t, min(TBLK, T - _t + 1)))
    _t += TBLK
NBLK = len(_BLOCKS)


def c_coeffs():
    """c_u for u=1..99 with mem2(99) = sum_u c_u * z2(u)."""
    u = np.arange(1, T + 1, dtype=np.float64)
    c = (ALPHA ** (T + 1 - u) - BETA ** (T + 1 - u)) / (ALPHA - BETA)
    return c.astype(np.float32)


def split_multi_waits(nc, max_waits=1):
    """This container's walrus rejects >1 semaphore wait per instruction;
    hoist excess waits onto same-engine NoOps inserted just before."""
    n = 0
    cnt = [0]
    for f in nc.m.functions:
        for bb in f.blocks:
            out = []
            changed = False
            for inst in bb.instructions:
                si = inst.sync_info
                waits = list(si.on_wait) if si is not None and si.on_wait else []
                if len(waits) > max_waits:
                    changed = True
                    n += 1
                    head, tail = waits[:-max_waits], waits[-max_waits:]
                    for i in range(0, len(head), max_waits):
                        nop = bass_rust.InstNoOp(
                            name=f"I-waitsplit-{cnt[0]}", ins=[], outs=[]
                        )
                        cnt[0] += 1
                        nop.engine = inst.engine
                        nop.sync_info = bass_rust.SyncInfo(
                            on_wait=head[i : i + max_waits], on_update=[]
                        )
                        out.append(nop)
                    si.on_wait = tail
                    inst.sync_info = si
                out.append(inst)
            if changed:
                bb.instructions = out
    return n


def build_nc():
    nc = bass.Bass(target_bir_lowering=False)

    x_d = nc.dram_tensor("inputs", [B_LOC, H_IN], F32, kind="ExternalInput")
    w0_d = nc.dram_tensor("W0", [H_IN, H1], F32, kind="ExternalInput")
    w1_d = nc.dram_tensor("W1", [H1, H2], F32, kind="ExternalInput")
    w2_d = nc.dram_tensor("W2", [H2, N_OUT], F32, kind="ExternalInput")
    cv_d = nc.dram_tensor("cvec", [128, T], F32, kind="ExternalInput")
    y_d = nc.dram_tensor("out", [B_LOC, N_OUT], F32, kind="ExternalOutput")

    from concourse.masks import make_identity

    with TileContext(nc) as tc:
        with (
            tc.tile_pool(name="consts", bufs=1) as consts,
            tc.tile_pool(name="tmp", bufs=1) as tmp,
            tc.tile_pool(name="state", bufs=1) as state,
            tc.tile_pool(name="s0p", bufs=2) as s0p,
            tc.tile_pool(name="s1p", bufs=2) as s1p,
            tc.tile_pool(name="z1sb", bufs=2) as z1sbp,
            tc.tile_pool(name="psA", bufs=1, space="PSUM") as psA,
            tc.tile_pool(name="psZ", bufs=2, space="PSUM") as psZ,
        ):
            # ---------------- setup ----------------
            # z0 path first: it gates the first layer-0 steps.
            x_sb = consts.tile([B_LOC, H_IN], F32)
            nc.sync.dma_start(out=x_sb, in_=x_d[:, :])
            w0_sb = consts.tile([128, 2, H1], F32)
            nc.sync.dma_start(
                out=w0_sb, in_=w0_d.rearrange("(kc p) j -> p kc j", p=128)
            )
            bias_m1 = consts.tile([128, 1], F32)
            nc.vector.memset(bias_m1, -1.0)
            ident = consts.tile([B_LOC, B_LOC], F32)
            make_identity(nc, ident)

            xt_sb = consts.tile([128, 2, B_LOC], F32)
            for kc in range(2):
                xt_ps = psA.tile([128, B_LOC], F32)
                nc.tensor.transpose(
                    xt_ps, x_sb[:, kc * 128 : (kc + 1) * 128], ident
                )
                nc.scalar.copy(xt_sb[:, kc, :], xt_ps)
            z0t = consts.tile([128, KC1, B_LOC], F32)
            z0_ps = psA.tile([128, KC1, B_LOC], F32)
            for jc in range(KC1):
                for kc in range(2):
                    nc.tensor.matmul(
                        z0_ps[:, jc, :],
                        w0_sb[:, kc, jc * 128 : (jc + 1) * 128],
                        xt_sb[:, kc, :],
                        start=(kc == 0),
                        stop=(kc == 1),
                    )
            nc.scalar.copy(z0t, z0_ps)

            # remaining weights/constants (needed from slot 1 onward)
            w1_sb = consts.tile([128, KC1, H2], F32)
            nc.sync.dma_start(
                out=w1_sb, in_=w1_d.rearrange("(kc p) j -> p kc j", p=128)
            )
            w1r = consts.tile([128, KC1, H2], F32R)
            nc.scalar.copy(w1r, w1_sb)
            w2_sb = consts.tile([128, KC1, N_OUT], F32)
            nc.sync.dma_start(
                out=w2_sb, in_=w2_d.rearrange("(kc p) o -> p kc o", p=128)
            )
            cb = consts.tile([128, T], F32)
            nc.sync.dma_start(out=cb, in_=cv_d[:, :])

            # W2c[p, kc, t, o] = W2[p, kc, o] * c[t], split hi+lo bf16.
            w2c_f = tmp.tile([128, KC1, T, N_OUT], F32)
            nc.gpsimd.tensor_tensor(
                out=w2c_f,
                in0=w2_sb.unsqueeze(2).to_broadcast([128, KC1, T, N_OUT]),
                in1=cb.unsqueeze(1).unsqueeze(3).to_broadcast([128, KC1, T, N_OUT]),
                op=Op.mult,
            )
            w2c_hi = consts.tile([128, KC1, T, N_OUT], BF16)
            nc.scalar.copy(w2c_hi, w2c_f)
            hi32 = tmp.tile([128, KC1, T, N_OUT], F32)
            nc.scalar.copy(hi32, w2c_hi)
            rem = tmp.tile([128, KC1, T, N_OUT], F32)
            nc.gpsimd.tensor_tensor(out=rem, in0=w2c_f, in1=hi32, op=Op.subtract)
            w2c_lo = consts.tile([128, KC1, T, N_OUT], BF16)
            nc.scalar.copy(w2c_lo, rem)

            # output accumulator (one PSUM bank, lives for the whole loop)
            out_ps = psA.tile([N_OUT, B_LOC], F32)

            # ---------------- state tiles ----------------
            m0 = state.tile([128, KC1, B_LOC], F32)
            bm0 = state.tile([128, KC1, B_LOC], F32)
            nm0a = state.tile([128, KC1, B_LOC], F32)
            nm0b = state.tile([128, KC1, B_LOC], F32)
            nm0s = [nm0a, nm0b]
            sga = state.tile([128, KC1, B_LOC], F32)
            sgb = state.tile([128, KC1, B_LOC], F32)
            sgs = [sga, sgb]
            mem1 = state.tile([128, KC1, B_LOC], F32)
            asyn = state.tile([128, KC1, B_LOC], F32)
            nsp1 = state.tile([128, KC1, B_LOC], F32)
            nm1a = state.tile([128, KC1, B_LOC], F32)
            nm1b = state.tile([128, KC1, B_LOC], F32)
            nm1s = [nm1a, nm1b]
            syn1a = state.tile([128, KC1, B_LOC], F32)
            syn1b = state.tile([128, KC1, B_LOC], F32)
            syn = [syn1a, syn1b]
            for st in (m0, bm0, mem1, syn1a, syn1b):
                nc.vector.memset(st, 0.0)

            s0_blks = [None] * NBLK
            s1_blks = [None] * NBLK
            z1_blks = [None] * NBLK
            z1_sbs = [None] * NBLK

            n_w2c = 2 * KC1 * T
            w2c_done = [0]

            def l0_step(blk, i):
                t0, tb = _BLOCKS[blk]
                t = t0 + i
                nm0 = nm0s[t % 2]
                # (a) nm0 = 0.9*m0 + z0          [VectorE]
                nc.vector.scalar_tensor_tensor(
                    out=nm0, in0=m0, scalar=BETA, in1=z0t,
                    op0=Op.mult, op1=Op.add,
                )
                # (b) m0 = (nm0<=1)*nm0          [VectorE]
                nc.vector.scalar_tensor_tensor(
                    out=m0, in0=nm0, scalar=1.0, in1=nm0,
                    op0=Op.is_le, op1=Op.mult,
                )

            def l0_spike(blk, i):
                # (c) s0 = relu(sign(nm0-1))     [ScalarE x2, lagged one step]
                t0, tb = _BLOCKS[blk]
                t = t0 + i
                nm0 = nm0s[t % 2]
                sg = sgs[t % 2]
                nc.scalar.activation(
                    out=sg, in_=nm0, func=Act.Sign, bias=bias_m1[:, 0:1], scale=1.0
                )
                nc.scalar.activation(
                    out=s0_blks[blk][:, :, i, :], in_=sg, func=Act.Relu
                )

            def l1_step(blk, i):
                t0, tb = _BLOCKS[blk]
                t = t0 + i
                p = t % 2
                nm1 = nm1s[p]
                # (d) syn = 0.95*syn_prev + z1; h-chunks 0-1 on VectorE,
                # h-chunks 2-3 via ScalarE scale + GpSimd add (z1 in SBUF)
                nc.vector.scalar_tensor_tensor(
                    out=syn[p][:, 0:2, :], in0=syn[1 - p][:, 0:2, :], scalar=ALPHA,
                    in1=z1_sbs[blk][:, 0:2, i, :],
                    op0=Op.mult, op1=Op.add,
                )
                nc.scalar.mul(out=asyn[:, 2:4, :], in_=syn[1 - p][:, 2:4, :], mul=ALPHA)
                nc.gpsimd.tensor_tensor(
                    out=syn[p][:, 2:4, :], in0=asyn[:, 2:4, :],
                    in1=z1_sbs[blk][:, 2:4, i, :], op=Op.add,
                )
                # (e) nm1 = 0.9*mem1 + syn       [VectorE]
                nc.vector.scalar_tensor_tensor(
                    out=nm1, in0=mem1, scalar=BETA, in1=syn[p],
                    op0=Op.mult, op1=Op.add,
                )
                # (f) mem1 = (nm1<=1)*nm1        [VectorE]
                nc.vector.scalar_tensor_tensor(
                    out=mem1, in0=nm1, scalar=1.0, in1=nm1,
                    op0=Op.is_le, op1=Op.mult,
                )

            def l1_spike(blk, i):
                # (g) s1 = (nm1>1) -> bf16       [GpSimd]
                t0, tb = _BLOCKS[blk]
                nm1 = nm1s[(t0 + i) % 2]
                nc.gpsimd.tensor_scalar(
                    out=s1_blks[blk][:, :, i, :], in0=nm1,
                    scalar1=1.0, scalar2=None, op0=Op.is_gt,
                )

            def w1_mms(blk):
                t0, tb = _BLOCKS[blk]
                for jc in range(KC1):
                    for kc in range(KC1):
                        nc.tensor.matmul(
                            z1_blks[blk][:, jc, 0:tb, :],
                            w1r[:, kc, jc * 128 : (jc + 1) * 128],
                            s0_blks[blk][:, kc, 0:tb, :],
                            start=(kc == 0),
                            stop=(kc == KC1 - 1),
                        )

            def w2c_mms(blk):
                t0, tb = _BLOCKS[blk]
                for i in range(tb):
                    t = t0 + i
                    for kc in range(KC1):
                        for wt in (w2c_hi, w2c_lo):
                            nc.tensor.matmul(
                                out_ps[:, :],
                                wt[:, kc, t - 1, :],
                                s1_blks[blk][:, kc, i, :],
                                start=(w2c_done[0] == 0),
                                stop=(w2c_done[0] == n_w2c - 1),
                                skip_group_check=True,
                            )
                            w2c_done[0] += 1

            # ------------- software-pipelined main loop -------------
            # slot sl: L0 on block sl, W1 matmul on block sl-1,
            #          L1 + W2c on block sl-2.
            for sl in range(NBLK + 2):
                lead = sl if sl < NBLK else None          # L0 block
                mmb = sl - 1 if 0 <= sl - 1 < NBLK else None
                lag = sl - 2 if sl - 2 >= 0 else None     # L1 block

                if lead is not None:
                    s0_blks[lead] = s0p.tile([128, KC1, TBLK, B_LOC], F32R, name="s0blk", tag="s0blk")
                    s1_blks[lead] = s1p.tile([128, KC1, TBLK, B_LOC], BF16, name="s1blk", tag="s1blk")
                if mmb is not None:
                    z1_blks[mmb] = psZ.tile([128, KC1, TBLK, B_LOC], F32, name="z1blk", tag="z1blk")
                    w1_mms(mmb)
                    z1_sbs[mmb] = z1sbp.tile(
                        [128, KC1, TBLK, B_LOC], F32, name="z1sb", tag="z1sb"
                    )
                    tbm = _BLOCKS[mmb][1]
                    nc.scalar.copy(
                        out=z1_sbs[mmb][:, :, 0:tbm, :],
                        in_=z1_blks[mmb][:, :, 0:tbm, :],
                    )

                tb0 = _BLOCKS[lead][1] if lead is not None else 0
                tb2 = _BLOCKS[lag][1] if lag is not None else 0
                for i in range(TBLK):
                    if lead is not None and i < tb0:
                        l0_step(lead, i)
                    if lead is not None and 0 <= i - 1 < tb0:
                        l0_spike(lead, i - 1)
                    if lag is not None and 0 <= i - 1 < tb2:
                        l1_spike(lag, i - 1)
                    if lag is not None and i < tb2:
                        l1_step(lag, i)
                if lead is not None:
                    l0_spike(lead, tb0 - 1)
                if lag is not None:
                    l1_spike(lag, tb2 - 1)
                    w2c_mms(lag)
                    z1_blks[lag] = None
                    z1_sbs[lag] = None
                    s0_blks[lag] = None
                    s1_blks[lag] = None

            # ---------------- epilogue ----------------
            out_sb = consts.tile([N_OUT, B_LOC], F32)
            nc.vector.tensor_copy(out_sb, out_ps)
            nc.sync.dma_start(out=y_d[:, :].transpose([1, 0]), in_=out_sb)

    return nc


_CACHE = {}


def _get_nc():
    if "nc" not in _CACHE:
        nc = build_nc()
        split_multi_waits(nc)
        _CACHE["nc"] = nc
    return _CACHE["nc"]


def make_in_maps(inputs, W0, W1, W2):
    cv = np.ascontiguousarray(
        np.broadcast_to(c_coeffs()[None, :], (128, T))
    ).astype(np.float32)
    maps = []
    for c in range(N_CORES):
        maps.append(
            {
                "inputs": np.ascontiguousarray(
                    inputs[c * B_LOC : (c + 1) * B_LOC]
                ).astype(np.float32),
                "W0": np.ascontiguousarray(W0).astype(np.float32),
                "W1": np.ascontiguousarray(W1).astype(np.float32),
                "W2": np.ascontiguousarray(W2).astype(np.float32),
                "cvec": cv,
            }
        )
    return maps


def kernel(inputs, W0, W1, W2):
    import os

    os.environ["BASS_NEVER_TRACE"] = "1"
    from concourse.bass_utils import run_bass_kernel_spmd

    nc = _get_nc()
    in_maps = make_in_maps(inputs, W0, W1, W2)
    res = run_bass_kernel_spmd(nc, in_maps, core_ids=list(range(N_CORES)))
    return np.concatenate([r["out"] for r in res.results], axis=0)
